# revision 1
# baseline (speedup 1.0000x reference)
"""Trainium2 kernel for nn_Attend_13537736916998 (sparse_attention).

Mathematical reduction of the reference:
  - sim = -max(||q_i||^2 + ||k_j||^2 - 2 q.k, 0) * D^-0.5 is <= 0 everywhere
    (masked entries are -FLT_MAX), so the selective-attention gate
    relu(sim[:, 0]) is identically zero for ALL inputs -> the gate/cumsum
    branch is a numerical no-op.
  - attn = hard + soft - stop_gradient(soft) evaluates elementwise to
    ((hard + soft) - soft): exactly 0 off the argmax and 1 + O(2^-24) at the
    argmax.  Hence out[b,h,i,:] = v[b,h, argmax_j sim[b,h,i,j], :] up to
    ~1e-7 relative error (verified: absmax 4.8e-7 vs the full reference).
  - argmax_j sim = argmax_{j<=i} (q_i . k_j - 0.5||k_j||^2)  (q_sq is a
    per-row constant and -d2*scale is monotone in d2; the max(.,0) clamp
    never binds for distinct random vectors).

Device kernel per NeuronCore (2 of the 16 (b,h) pairs each):
  S[i,j] = [q_i, 1] . [k_j, -0.5||k_j||^2]   via one 65-contraction matmul
  causal argmax per row (vector-engine max8 + max_index)
  out rows gathered from v in HBM via indirect DMA.

Inputs are pre-transposed on the host (layout marshalling only): qa[t] is
[65, 2048] = [q^T; ones], kt[t] is [64, 2048] = k^T.  The -0.5||k||^2 row
is computed on device.  Output is emitted in gather layout [2, 128, 16, 64]
(partition-major) and re-ordered on the host during unsharding.
"""

import numpy as np
from contextlib import ExitStack

import concourse.bass as bass
import concourse.bacc as bacc
import concourse.tile as tile
from concourse import mybir
import concourse.bass_utils as bass_utils

B, H, N, D = 2, 8, 2048, 64
P = 128
NT = N // P            # 16 row tiles per (b,h) pair
T = 2                  # (b,h) pairs per core
NCORES = 8
NEG = -1.0e30
F32 = mybir.dt.float32
F32R = mybir.dt.float32r
U32 = mybir.dt.uint32

# float32r (TF32-like rounded fp32, 1 cyc/row) is too imprecise for the
# argmax: walrus requires producers to round outputs to fp32r, i.e. it is a
# reduced-precision format.  Plain fp32 (4 cyc/row) keeps argmax fidelity.
USE_F32R = False


def _mm_cast(ap):
    return ap.bitcast(F32R) if USE_F32R else ap


def kernel_body(tc, qa, kt, v, out):
    nc = tc.nc
    with ExitStack() as ctx:
        consts = ctx.enter_context(tc.tile_pool(name="consts", bufs=1))
        io = ctx.enter_context(tc.tile_pool(name="io", bufs=2))
        work = ctx.enter_context(tc.tile_pool(name="work", bufs=4))
        outp = ctx.enter_context(tc.tile_pool(name="outp", bufs=2))
        small = ctx.enter_context(tc.tile_pool(name="small", bufs=6))
        ps_pool = ctx.enter_context(tc.tile_pool(name="ps", bufs=7, space="PSUM"))
        psk_pool = ctx.enter_context(tc.tile_pool(name="psk", bufs=1, space="PSUM"))

        ones_col = consts.tile([D, 1], F32)
        nc.vector.memset(ones_col, 1.0)

        for t in range(T):
            # chunked input DMAs + ksq pipeline so the first S-matmul can
            # start as soon as kt cols [0:512] (and their ksq row) land.
            qa_t = io.tile([D + 1, N], F32, tag="qa")
            for c in reversed(range(N // 512)):  # row tiles run largest-first
                nc.sync.dma_start(
                    out=qa_t[:, c * 512:(c + 1) * 512],
                    in_=qa[t][:, c * 512:(c + 1) * 512],
                )
            kt_t = io.tile([D + 1, N], F32, tag="kt")
            sq = io.tile([D, N], F32, tag="sq")
            for c in range(N // 512):
                cs = slice(c * 512, (c + 1) * 512)
                nc.sync.dma_start(out=kt_t[0:D, cs], in_=kt[t][:, cs])
                # kt_t row 64 <- -0.5 * sum_d k[d,j]^2
                nc.scalar.square(sq[:, cs], kt_t[0:D, cs])
                pk = psk_pool.tile([1, 512], F32, tag="pk")
                nc.tensor.matmul(
                    pk,
                    lhsT=_mm_cast(ones_col),
                    rhs=_mm_cast(sq[:, cs]),
                    start=True,
                    stop=True,
                )
                nc.scalar.mul(kt_t[D:D + 1, cs], pk, -0.5)

            idxs = outp.tile([P, NT, 8], U32, tag="idxs")
            vout = outp.tile([P, NT, D], F32, tag="vout")
            # big/small interleave: PE stays fed with large tiles while the
            # vector engine's backlog drains on small ones; the pair ends on
            # the cheapest tiles so the end-of-kernel tail is short.
            order = []
            lo_m, hi_m = 0, NT - 1
            while hi_m >= lo_m:
                order.append(hi_m); hi_m -= 1
                if hi_m >= lo_m:
                    order.append(lo_m); lo_m += 1
            # order = [15, 0, 14, 1, ..., 8, 7]
            for m in order:
                W = (m + 1) * P
                S = work.tile([P, N], F32, tag="S")
                nchunks = (W + 511) // 512
                for c in range(nchunks):
                    lo = c * 512
                    hi = min(W, lo + 512)
                    ps = ps_pool.tile([P, 512], F32, tag="ps")
                    nc.tensor.matmul(
                        ps[:, : hi - lo],
                        lhsT=_mm_cast(qa_t[:, m * P:(m + 1) * P]),
                        rhs=_mm_cast(kt_t[:, lo:hi]),
                        start=True,
                        stop=True,
                    )
                    nc.scalar.copy(S[:, lo:hi], ps[:, : hi - lo])
                # causal mask inside the diagonal 128x128 block:
                # keep column f (global j = m*P+f) for row p iff p - f >= 0
                nc.gpsimd.affine_select(
                    out=S[:, W - P:W],
                    in_=S[:, W - P:W],
                    pattern=[[-1, P]],
                    base=0,
                    channel_multiplier=1,
                    compare_op=mybir.AluOpType.is_ge,
                    fill=NEG,
                )
                mx = small.tile([P, 8], F32, tag="mx")
                nc.vector.max(mx, S[:, 0:W])
                nc.vector.max_index(idxs[:, m, :], mx, S[:, 0:W])
                # gather the 128 winning v rows for this row tile.
                # NB: one offset column per indirect DMA — multi-column offset
                # tables mis-generate descriptors on HW.
                nc.gpsimd.indirect_dma_start(
                    out=vout[:, m, :],
                    out_offset=None,
                    in_=v,
                    in_offset=bass.IndirectOffsetOnAxis(ap=idxs[:, m, 0:1], axis=1),
                    element_offset=t * N * D,
                )

            # two half-writes: the m=15..8 gathers finish long before m=7..0
            nc.sync.dma_start(out=out[t][:, 8:NT, :], in_=vout[:, 8:NT, :])
            nc.sync.dma_start(out=out[t][:, 0:8, :], in_=vout[:, 0:8, :])


_NC_CACHE = None


def build_nc():
    global _NC_CACHE
    if _NC_CACHE is not None:
        return _NC_CACHE
    nc = bacc.Bacc(
        "TRN2",
        target_bir_lowering=False,
        debug=False,
        enable_asserts=False,
        num_devices=NCORES,
    )
    qa = nc.dram_tensor("qa", [T, D + 1, N], F32, kind="ExternalInput").ap()
    kt = nc.dram_tensor("kt", [T, D, N], F32, kind="ExternalInput").ap()
    v = nc.dram_tensor("v", [T, N, D], F32, kind="ExternalInput").ap()
    out = nc.dram_tensor("out", [T, P, NT, D], F32, kind="ExternalOutput").ap()
    with tile.TileContext(nc) as tc:
        kernel_body(tc, qa, kt, v, out)
    nc.compile()
    _NC_CACHE = nc
    return nc


def make_in_maps(q, k, v):
    q = np.asarray(q, dtype=np.float32)
    k = np.asarray(k, dtype=np.float32)
    v = np.asarray(v, dtype=np.float32)
    assert q.shape == (B, H, N, D), q.shape
    in_maps = []
    for c in range(NCORES):
        qa_c = np.empty((T, D + 1, N), np.float32)
        kt_c = np.empty((T, D, N), np.float32)
        v_c = np.empty((T, N, D), np.float32)
        for t in range(T):
            gp = T * c + t
            b, h = divmod(gp, H)
            qa_c[t, :D] = q[b, h].T
            qa_c[t, D] = 1.0
            kt_c[t] = k[b, h].T
            v_c[t] = v[b, h]
        in_maps.append({"qa": qa_c, "kt": kt_c, "v": v_c})
    return in_maps


def unmarshal(results):
    out = np.empty((B, H, N, D), np.float32)
    for c in range(NCORES):
        o = np.asarray(results[c]["out"])  # [T, P, NT, D]
        for t in range(T):
            gp = T * c + t
            b, h = divmod(gp, H)
            out[b, h] = o[t].transpose(1, 0, 2).reshape(N, D)
    return out


def kernel(q, k, v):
    nc = build_nc()
    in_maps = make_in_maps(q, k, v)
    res = bass_utils.run_bass_kernel_spmd(nc, in_maps, core_ids=list(range(NCORES)))
    return unmarshal(res.results)



# revision 3
# speedup vs baseline: 1.1973x; 1.1973x over previous
"""Trainium2 kernel for nn_Attend_13537736916998 (sparse_attention).

Mathematical reduction of the reference:
  - sim = -max(||q_i||^2 + ||k_j||^2 - 2 q.k, 0) * D^-0.5 is <= 0 everywhere
    (masked entries are -FLT_MAX), so the selective-attention gate
    relu(sim[:, 0]) is identically zero for ALL inputs -> the gate/cumsum
    branch is a numerical no-op.
  - attn = hard + soft - stop_gradient(soft) evaluates elementwise to
    ((hard + soft) - soft): exactly 0 off the argmax and 1 + O(2^-24) at the
    argmax.  Hence out[b,h,i,:] = v[b,h, argmax_j sim[b,h,i,j], :] up to
    ~1e-7 relative error.
  - argmax_j sim = argmax_{j<=i} (q_i . k_j - 0.5||k_j||^2)  (q_sq is a
    per-row constant and -d2*scale is monotone in d2; the max(.,0) clamp
    never binds for distinct random vectors).

Score matmul runs as an exact-enough 2-pass fp16 limb decomposition
(1 cyc/row/pass on the PE instead of fp32's 4):
  qhi = fp16(q),  qlo = fp16(q - qhi)        (q accurate to ~2^-22)
  khi = fp16(k),  klo = fp16(k - khi)
  b   = -0.5||k||^2 (fp32 via PE),  b1 = fp16(b), b2 = fp16(b - b1)
  pass1: [qhi; qlo]^T @ [khi; khi]  = (qhi+qlo).khi
  pass2: [qhi; 1; 1]^T @ [klo; b1; b2] = qhi.klo + b
  sum   = q.k - 0.5||k||^2 + O(1e-5)   (dropped qlo.klo ~ 2^-24)
The O(1e-5) matches fp32's own rounding noise; argmax agreement with the
fp32 reference is verified on the fixed harness input.

Device kernel per NeuronCore (2 of the 16 (b,h) pairs each):
  causal argmax per row (vector-engine max8 + max_index)
  out rows gathered from v in HBM via indirect DMA.

Output is emitted in gather layout [2, 128, 16, 64] (partition-major) and
re-ordered on the host during unsharding.
"""

import numpy as np
from contextlib import ExitStack

import concourse.bass as bass
import concourse.bacc as bacc
import concourse.tile as tile
from concourse import mybir
import concourse.bass_utils as bass_utils

B, H, N, D = 2, 8, 2048, 64
P = 128
NT = N // P            # 16 row tiles per (b,h) pair
T = 2                  # (b,h) pairs per core
NCORES = 8
NEG = -1.0e30
F32 = mybir.dt.float32
F16 = mybir.dt.float16
U32 = mybir.dt.uint32


def kernel_body(tc, qa, kt, v, out):
    nc = tc.nc
    with ExitStack() as ctx:
        consts = ctx.enter_context(tc.tile_pool(name="consts", bufs=1))
        io = ctx.enter_context(tc.tile_pool(name="io", bufs=2))
        work = ctx.enter_context(tc.tile_pool(name="work", bufs=4))
        outp = ctx.enter_context(tc.tile_pool(name="outp", bufs=2))
        small = ctx.enter_context(tc.tile_pool(name="small", bufs=6))
        ps_pool = ctx.enter_context(tc.tile_pool(name="ps", bufs=7, space="PSUM"))
        psk_pool = ctx.enter_context(tc.tile_pool(name="psk", bufs=1, space="PSUM"))

        ones_col = consts.tile([D, 1], F32)
        nc.vector.memset(ones_col, 1.0)

        for t in range(T):
            # ---- q-side prep: fp32 load + fp16 limb split, high cols first
            # (row tiles are processed largest-first, so lhsT slices for
            # tiles 15..12 are needed first).
            qa_t = io.tile([D + 1, N], F32, tag="qa")
            qhl = io.tile([2 * D, N], F16, tag="qhl")      # [qhi; qlo]
            qho = io.tile([D + 2, N], F16, tag="qho")      # [qhi; 1; 1]
            nc.vector.memset(qho[D:D + 2, :], 1.0)
            for c in reversed(range(N // 512)):
                cs = slice(c * 512, (c + 1) * 512)
                nc.sync.dma_start(out=qa_t[:, cs], in_=qa[t][:, cs])
                nc.scalar.copy(qhl[0:D, cs], qa_t[0:D, cs])            # qhi
                nc.vector.tensor_sub(qhl[D:2 * D, cs], qa_t[0:D, cs],
                                     qhl[0:D, cs])                     # qlo
                nc.vector.tensor_copy(qho[0:D, cs], qhl[0:D, cs])      # qhi dup

            # ---- k-side prep: fp16 limbs + fp32 ksq bias (split to fp16)
            kt_t = io.tile([D, N], F32, tag="kt")
            sq = io.tile([D, N], F32, tag="sq")
            khh = io.tile([2 * D, N], F16, tag="khh")      # [khi; khi]
            klb = io.tile([D + 2, N], F16, tag="klb")      # [klo; b1; b2]
            b32 = io.tile([1, N], F32, tag="b32")
            # bias limbs staged side-by-side on partition 0 (engine writes
            # must start at a multiple-of-32 partition), then DMA'd into
            # klb partitions 64:66.
            bb = io.tile([1, 2, N], F16, tag="bb")
            for c in range(N // 512):
                cs = slice(c * 512, (c + 1) * 512)
                nc.sync.dma_start(out=kt_t[:, cs], in_=kt[t][:, cs])
                nc.scalar.copy(khh[0:D, cs], kt_t[:, cs])              # khi
                nc.vector.tensor_sub(klb[0:D, cs], kt_t[:, cs],
                                     khh[0:D, cs])                     # klo
                nc.vector.tensor_copy(khh[D:2 * D, cs], khh[0:D, cs])  # khi dup
                # bias row: b = -0.5 * sum_d k[d,j]^2 in fp32, then 2-limb fp16
                nc.scalar.square(sq[:, cs], kt_t[:, cs])
                pk = psk_pool.tile([1, 512], F32, tag="pk")
                nc.tensor.matmul(pk, lhsT=ones_col, rhs=sq[:, cs],
                                 start=True, stop=True)
                nc.scalar.mul(b32[:, cs], pk, -0.5)
                nc.scalar.copy(bb[:, 0, cs], b32[:, cs])               # b1
                nc.vector.tensor_sub(bb[:, 1, cs], b32[:, cs],
                                     bb[:, 0, cs])                     # b2
                nc.sync.dma_start(out=klb[D:D + 2, cs], in_=bb[:, :, cs])

            idxs = outp.tile([P, NT, 8], U32, tag="idxs")
            vout = outp.tile([P, NT, D], F32, tag="vout")
            # big/small interleave: PE stays fed with large tiles while the
            # vector engine's backlog drains on small ones; the pair ends on
            # the cheapest tiles so the end-of-kernel tail is short.
            order = []
            lo_m, hi_m = 0, NT - 1
            while hi_m >= lo_m:
                order.append(hi_m); hi_m -= 1
                if hi_m >= lo_m:
                    order.append(lo_m); lo_m += 1
            # order = [15, 0, 14, 1, ..., 8, 7]
            for m in order:
                W = (m + 1) * P
                ms = slice(m * P, (m + 1) * P)
                S = work.tile([P, N], F32, tag="S")
                nchunks = (W + 511) // 512
                for c in range(nchunks):
                    lo = c * 512
                    hi = min(W, lo + 512)
                    ps = ps_pool.tile([P, 512], F32, tag="ps")
                    nc.tensor.matmul(ps[:, : hi - lo], lhsT=qhl[:, ms],
                                     rhs=khh[:, lo:hi], start=True, stop=False)
                    nc.tensor.matmul(ps[:, : hi - lo], lhsT=qho[:, ms],
                                     rhs=klb[:, lo:hi], start=False, stop=True)
                    nc.scalar.copy(S[:, lo:hi], ps[:, : hi - lo])
                # causal mask inside the diagonal 128x128 block:
                # keep column f (global j = m*P+f) for row p iff p - f >= 0
                nc.gpsimd.affine_select(
                    out=S[:, W - P:W],
                    in_=S[:, W - P:W],
                    pattern=[[-1, P]],
                    base=0,
                    channel_multiplier=1,
                    compare_op=mybir.AluOpType.is_ge,
                    fill=NEG,
                )
                mx = small.tile([P, 8], F32, tag="mx")
                nc.vector.max(mx, S[:, 0:W])
                nc.vector.max_index(idxs[:, m, :], mx, S[:, 0:W])
                # gather the 128 winning v rows for this row tile.
                # NB: one offset column per indirect DMA — multi-column offset
                # tables mis-generate descriptors on HW.
                nc.gpsimd.indirect_dma_start(
                    out=vout[:, m, :],
                    out_offset=None,
                    in_=v,
                    in_offset=bass.IndirectOffsetOnAxis(ap=idxs[:, m, 0:1], axis=1),
                    element_offset=t * N * D,
                )

            # two half-writes: the m=15..8 gathers finish long before m=7..0
            nc.sync.dma_start(out=out[t][:, 8:NT, :], in_=vout[:, 8:NT, :])
            nc.sync.dma_start(out=out[t][:, 0:8, :], in_=vout[:, 0:8, :])


_NC_CACHE = None


def build_nc():
    global _NC_CACHE
    if _NC_CACHE is not None:
        return _NC_CACHE
    nc = bacc.Bacc(
        "TRN2",
        target_bir_lowering=False,
        debug=False,
        enable_asserts=False,
        num_devices=NCORES,
    )
    qa = nc.dram_tensor("qa", [T, D + 1, N], F32, kind="ExternalInput").ap()
    kt = nc.dram_tensor("kt", [T, D, N], F32, kind="ExternalInput").ap()
    v = nc.dram_tensor("v", [T, N, D], F32, kind="ExternalInput").ap()
    out = nc.dram_tensor("out", [T, P, NT, D], F32, kind="ExternalOutput").ap()
    with tile.TileContext(nc) as tc:
        kernel_body(tc, qa, kt, v, out)
    nc.compile()
    _NC_CACHE = nc
    return nc


def make_in_maps(q, k, v):
    q = np.asarray(q, dtype=np.float32)
    k = np.asarray(k, dtype=np.float32)
    v = np.asarray(v, dtype=np.float32)
    assert q.shape == (B, H, N, D), q.shape
    in_maps = []
    for c in range(NCORES):
        qa_c = np.empty((T, D + 1, N), np.float32)
        kt_c = np.empty((T, D, N), np.float32)
        v_c = np.empty((T, N, D), np.float32)
        for t in range(T):
            gp = T * c + t
            b, h = divmod(gp, H)
            qa_c[t, :D] = q[b, h].T
            qa_c[t, D] = 1.0
            kt_c[t] = k[b, h].T
            v_c[t] = v[b, h]
        in_maps.append({"qa": qa_c, "kt": kt_c, "v": v_c})
    return in_maps


def unmarshal(results):
    out = np.empty((B, H, N, D), np.float32)
    for c in range(NCORES):
        o = np.asarray(results[c]["out"])  # [T, P, NT, D]
        for t in range(T):
            gp = T * c + t
            b, h = divmod(gp, H)
            out[b, h] = o[t].transpose(1, 0, 2).reshape(N, D)
    return out


def kernel(q, k, v):
    nc = build_nc()
    in_maps = make_in_maps(q, k, v)
    res = bass_utils.run_bass_kernel_spmd(nc, in_maps, core_ids=list(range(NCORES)))
    return unmarshal(res.results)
